# revision 27
# baseline (speedup 1.0000x reference)
"""Causal self-attention (B=2, T=2048, E=1024, H=16, D=64) on 8 TRN2 cores.

Sharding: core = (batch, head-group): b = core // 4, heads 4g..4g+3 with
g = core % 4 (data parallel over batch x tensor parallel over heads).
Each core computes qkv projection for its 4 heads, causal attention, and
a partial output projection (its head rows of w_proj). Host sums the 4
partials per batch and adds b_proj.

All device compute is bf16 into fp32 PSUM (tolerance is 2e-2; bf16
matmuls run 1 col/cycle vs ~3 cycles/col for fp32 on TRN2's PE).

Device layout (per core):
  inputs (bf16): xt [1024, 2048] = x[b].T
                 wqk [1024, 512] = [w_q cols | w_k cols] for the 4 heads
                 wv  [1024, 256]
                 wp  [256, 1024] = w_proj rows for the 4 heads
                 masks [128, 4, 512] causal masks (diagonal positions)
  output (f32):  out [2048, 1024] partial projection

Matmul chain (no transposes needed):
  QK^T [512, 2048] = (x @ wqk)^T   via lhsT=wqk chunk, rhs=xt chunk
  V    [2048, 256+ones]            via lhsT=xt chunk, rhs=wv chunk
  S^T  [k,q] tiles                 via lhsT=K^T slice, rhs=Q^T slice
  O^T+sums = [V|1]^T P             via lhsT=v tile, rhs=exp(S^T) tile
  out  [2048,1024]                 via lhsT=attn^T chunk, rhs=wp chunk
Normalized O^T goes to a separate attn buffer (not over Q^T rows) so no
write-after-read hazard serializes consecutive attention units.

Scheduling: attention runs in four two-head units. The two heads of a
unit sit on partitions 0-63 / 64-127, so their K=64-contraction S^T
matmuls pack into disjoint row-strips of the PE array and execute
concurrently. Each unit is a single stream with lag-1 PV (PV of step
s-1 issues while exp(s) runs on ACT), and pure-PE "filler" work (v/qk
projection groups, out-proj tiles) is interleaved at step boundaries,
emitted before the lagged PV so unit-entry stalls get filled. Fillers
are placed so no filler produces data an earlier-emitted instruction
consumes, and only where their dependencies are already met (a waiting
filler blocks the in-order PE stream).
"""

import sys

sys.path.insert(0, "/opt/trn_rl_repo")

import numpy as np
import ml_dtypes

import concourse.bacc as bacc
import concourse.tile as tile
from concourse import mybir
from concourse.bass_utils import run_bass_kernel_spmd

F32 = mybir.dt.float32
BF16 = mybir.dt.bfloat16
Exp = mybir.ActivationFunctionType.Exp

B, T, E = 2, 2048, 1024
H, D = 16, 64
NCORES = 8
HPC = 4          # heads per core
QC = HPC * D     # 256 q cols per core
P = 128

_PROG = None


def _build():
    nc = bacc.Bacc("TRN2", target_bir_lowering=False, debug=False)

    xt_d = nc.dram_tensor("xt", [E, T], BF16, kind="ExternalInput")
    wqk_d = nc.dram_tensor("wqk", [E, 2 * QC], BF16, kind="ExternalInput")
    wv_d = nc.dram_tensor("wv", [E, QC], BF16, kind="ExternalInput")
    wp_d = nc.dram_tensor("wp", [QC, E], BF16, kind="ExternalInput")
    mk_d = nc.dram_tensor("masks", [P, 4, 512], BF16, kind="ExternalInput")
    out_d = nc.dram_tensor("out", [T, E], F32, kind="ExternalOutput")

    KC = E // P       # 8 contraction chunks over E
    NT = T // P       # 16 T tiles of 128

    with tile.TileContext(nc) as tc:
        with (
            tc.tile_pool(name="persist", bufs=1) as persist,
            tc.tile_pool(name="inp", bufs=1) as inp,
            tc.tile_pool(name="pt", bufs=4) as ptp,
            tc.tile_pool(name="small", bufs=2) as small,
            tc.tile_pool(name="denp", bufs=2) as denp,
            tc.tile_pool(name="stage", bufs=3) as stg,
            tc.tile_pool(name="big", bufs=2, space="PSUM") as big,
            tc.tile_pool(name="po", bufs=1, space="PSUM") as pop,
        ):
            # ---- persistent sbuf ----
            qk_sb = [persist.tile([P, T], BF16, name=f"qk{m}") for m in range(4)]
            at_sb = [persist.tile([P, T], BF16, name=f"at{c}") for c in range(2)]
            v_sb = [persist.tile([P, HPC, D + 1], BF16, name=f"v{t}") for t in range(NT)]
            mask_sb = persist.tile([P, 4, 512], BF16, name="masks")
            wp_sb = [persist.tile([P, E], BF16, name=f"wp{c}") for c in range(2)]

            # ---- input DMAs (first-needed first) ----
            xt_sb = [inp.tile([P, T], BF16, name=f"xt{c}") for c in range(KC)]
            wqk_sb = [inp.tile([P, 2 * QC], BF16, name=f"wqk{c}") for c in range(KC)]
            wv_sb = [inp.tile([P, QC], BF16, name=f"wv{c}") for c in range(KC)]
            for c in range(KC):
                nc.sync.dma_start(out=wv_sb[c], in_=wv_d[c * P : (c + 1) * P, :])
                nc.sync.dma_start(out=wqk_sb[c], in_=wqk_d[c * P : (c + 1) * P, :])
                nc.sync.dma_start(
                    out=xt_sb[c][:, 0:1024], in_=xt_d[c * P : (c + 1) * P, 0:1024]
                )
            nc.sync.dma_start(out=mask_sb, in_=mk_d[:])
            for c in range(KC):
                nc.sync.dma_start(
                    out=xt_sb[c][:, 1024:2048],
                    in_=xt_d[c * P : (c + 1) * P, 1024:2048],
                )
            for c in range(2):
                nc.sync.dma_start(out=wp_sb[c], in_=wp_d[c * P : (c + 1) * P, :])

            ones4 = persist.tile([P, HPC, 1], F32, name="ones4")
            nc.vector.memset(ones4, 1.0)
            for t in range(NT):
                nc.vector.tensor_copy(v_sb[t][:, :, D : D + 1], ones4)

            def v_group(t, eng):
                ps = big.tile([P, 2, 512], F32, name="ps")
                for c in range(KC):
                    nc.tensor.matmul(
                        ps[:, 0, :QC],
                        lhsT=xt_sb[c][:, t * P : (t + 1) * P],
                        rhs=wv_sb[c],
                        start=(c == 0),
                        stop=(c == KC - 1),
                    )
                cp = nc.scalar.copy if eng == "s" else nc.vector.tensor_copy
                cp(
                    v_sb[t][:, :, 0:D],
                    ps[:, 0, :QC].rearrange("p (h d) -> p h d", h=HPC),
                )

            def qk_group(m, u, eng):
                # c outer / nl inner: consecutive matmuls share lhsT
                ps = big.tile([P, 2, 512], F32, name="ps")
                for c in range(KC):
                    for nl in range(2):
                        nc.tensor.matmul(
                            ps[:, nl, :],
                            lhsT=wqk_sb[c][:, m * P : (m + 1) * P],
                            rhs=xt_sb[c][:, (2 * u + nl) * 512 : (2 * u + nl + 1) * 512],
                            start=(c == 0),
                            stop=(c == KC - 1),
                        )
                cp = nc.scalar.copy if eng == "s" else nc.vector.tensor_copy
                cp(
                    qk_sb[m][:, u * 1024 : (u + 1) * 1024],
                    ps.rearrange("p a b -> p (a b)"),
                )

            def out_tile(t, eng):
                ps = big.tile([P, 2, 512], F32, name="ps")
                for c in range(2):
                    for nl in range(2):
                        nc.tensor.matmul(
                            ps[:, nl, :],
                            lhsT=at_sb[c][:, t * P : (t + 1) * P],
                            rhs=wp_sb[c][:, nl * 512 : (nl + 1) * 512],
                            start=(c == 0),
                            stop=(c == 1),
                        )
                st = stg.tile([P, 1024], F32, name="st")
                cp = nc.scalar.copy if eng == "s" else nc.vector.tensor_copy
                cp(st, ps.rearrange("p a b -> p (a b)"))
                nc.sync.dma_start(out=out_d[t * P : (t + 1) * P, :], in_=st)

            def attn_unit(po, p, fillers=()):
                """Attention for heads (2po, 2po+1), query block p.

                The two heads live on partitions 0-63 / 64-127 of qk_sb[po]
                (Q^T) and qk_sb[2+po] (K^T). Their S^T matmuls have K=64
                contraction, so emitting them back-to-back packs them into
                disjoint row-strips of the PE array (row_grp auto-derived
                from the lhsT base partition) and they run concurrently.
                Per step (k-tile i, 512-col q-span jj) the S^T pair lands in
                one [128, 2, 512] PSUM tile (head per bank), one ACT exp
                covers both banks, then per-head PV accumulates. PV is
                emitted with lag 1 so the PE consumes step s-1 while ACT
                runs exp(s). `fillers` = (slot, closure) PE-only work.
                """
                Qh = qk_sb[po]
                Kh = qk_sb[2 + po]
                qbase = p * 1024
                psot = pop.tile([D + 1, 2, 2, 512], F32, name="pso")
                pso = [psot[:, 0], psot[:, 1]]
                nk = 8 * p + 8
                fills = {s: f for s, f in fillers}
                # steps: (i, jj, w) with w = causal trim start within the span
                steps = []
                for i in range(nk):
                    for jj in range(2):
                        m = i - 8 * p - 4 * jj
                        if m > 3:
                            continue
                        steps.append((i, jj, max(0, 128 * m), m))
                jlast = {jj: max(i for i, j, _, _ in steps if j == jj) for jj in range(2)}

                def emit_pv(i, jj, w, pt):
                    for hh in range(2):
                        nc.tensor.matmul(
                            pso[hh][:, jj, w:512],
                            lhsT=v_sb[i][:, 2 * po + hh, :],
                            rhs=pt[:, hh, w:512],
                            start=(i == 0),
                            stop=(i == jlast[jj]),
                        )

                pend = None
                for s, (i, jj, w, m) in enumerate(steps):
                    ps = big.tile([P, 2, 512], F32, name="ps")
                    pt = ptp.tile([P, 2, 512], BF16, name="pt")
                    q0 = qbase + jj * 512
                    # packed S^T pair: row strips 0-63 / 64-127 run concurrently
                    for hh in range(2):
                        nc.tensor.matmul(
                            ps[:, hh, w:512],
                            lhsT=Kh[64 * hh : 64 * hh + 64, i * P : (i + 1) * P],
                            rhs=Qh[64 * hh : 64 * hh + 64, q0 + w : q0 + 512],
                            start=True,
                            stop=True,
                        )
                    # filler before the lagged PV: keeps the PE stream moving
                    # while pso (at unit entry) or exp results are pending
                    f = fills.get(s)
                    if f is not None:
                        f()
                    # lagged PV: PE consumes pt(s-1) while ACT runs exp(s)
                    if pend is not None:
                        emit_pv(*pend)
                    # one exp over both heads' banks
                    nc.scalar.activation(pt[:, :, w:512], ps[:, :, w:512], Exp, scale=0.125)
                    # causal mask on diagonal tiles
                    if m >= 0:
                        for hh in range(2):
                            nc.vector.tensor_mul(
                                pt[:, hh, w:512],
                                pt[:, hh, w:512],
                                mask_sb[:, m, w:512],
                            )
                    pend = (i, jj, w, pt)
                emit_pv(*pend)
                # normalize and write O^T to the attn buffer; jj-major so
                # the jj=0 span's writes land first (out tiles of that span
                # unblock before the jj=1 chains finish)
                dens = []
                for hh in range(2):
                    den = denp.tile([1, 2, 512], F32, name="den")
                    nc.scalar.copy(den, pso[hh][D : D + 1, :, :])
                    dens.append(den)
                for jj in range(2):
                    for hh in range(2):
                        rec = small.tile([1, 512], F32, name="rec")
                        rb = small.tile([64, 512], F32, name="rb")
                        nc.vector.reciprocal_approx_fast(out=rec, in_=dens[hh][:, jj, :])
                        nc.gpsimd.partition_broadcast(rb, rec)
                        nc.vector.tensor_mul(
                            at_sb[po][
                                64 * hh : 64 * hh + 64,
                                qbase + jj * 512 : qbase + (jj + 1) * 512,
                            ],
                            pso[hh][0:D, jj, :],
                            rb,
                        )

            # ---- emission: lead-in projections, then attention with fillers
            for t in range(8):
                v_group(t, "s")
            qk_group(0, 0, "s")
            qk_group(2, 0, "s")
            # U1 = heads 0,1 @ q 0..1023 (12 steps)
            attn_unit(
                0, 0,
                [(0, lambda: qk_group(1, 0, "v")), (1, lambda: qk_group(3, 0, "v"))]
                + [(6 + j, lambda t=t: v_group(t, "v")) for j, t in enumerate(range(8, 12))],
            )
            # U2 = heads 2,3 @ q 0..1023 (12 steps)
            attn_unit(
                1, 0,
                [(0, lambda: qk_group(0, 1, "v")), (1, lambda: qk_group(2, 1, "v"))]
                + [(5 + j, lambda t=t: v_group(t, "v")) for j, t in enumerate(range(12, 16))],
            )
            # U3 = heads 0,1 @ q 1024..2047 (28 steps)
            attn_unit(
                0, 1,
                [(0, lambda: qk_group(1, 1, "v")), (1, lambda: qk_group(3, 1, "v"))]
                + [(s, lambda t=t: out_tile(t, "v")) for s, t in zip((14, 18, 22, 25), range(0, 4))],
            )
            # U4 = heads 2,3 @ q 1024..2047 (28 steps)
            attn_unit(
                1, 1,
                [(0, lambda: out_tile(4, "v")), (1, lambda: out_tile(5, "v")),
                 (18, lambda: out_tile(6, "v")), (24, lambda: out_tile(7, "v"))],
            )
            for t in range(8, NT):
                out_tile(t, "s" if t % 2 else "v")

    nc.compile()
    return nc


def _get_prog():
    global _PROG
    if _PROG is None:
        _PROG = _build()
    return _PROG


def _masks_np():
    kk = np.arange(P)[:, None]
    qq = np.arange(512)[None, :]
    return np.stack(
        [((128 * m + kk) <= qq) for m in range(4)], axis=1
    ).astype(ml_dtypes.bfloat16)


def _bf(a):
    return np.ascontiguousarray(a).astype(ml_dtypes.bfloat16)


def _shard(x, w_qkv, w_proj):
    masks = _masks_np()
    in_maps = []
    for core in range(NCORES):
        b, g = core // HPC, core % HPC
        c0 = g * QC
        in_maps.append(
            {
                "xt": _bf(x[b].T),
                "wqk": _bf(
                    np.concatenate(
                        [w_qkv[:, c0 : c0 + QC], w_qkv[:, E + c0 : E + c0 + QC]],
                        axis=1,
                    )
                ),
                "wv": _bf(w_qkv[:, 2 * E + c0 : 2 * E + c0 + QC]),
                "wp": _bf(w_proj[c0 : c0 + QC, :]),
                "masks": masks,
            }
        )
    return in_maps


def _run(inputs, **kwargs):
    x = np.asarray(inputs["x"], dtype=np.float32)
    w_qkv = np.asarray(inputs["w_qkv"], dtype=np.float32)
    w_proj = np.asarray(inputs["w_proj"], dtype=np.float32)
    b_proj = np.asarray(inputs["b_proj"], dtype=np.float32)

    nc = _get_prog()
    in_maps = _shard(x, w_qkv, w_proj)
    res = run_bass_kernel_spmd(nc, in_maps, core_ids=list(range(NCORES)), **kwargs)

    out = np.zeros((B, T, E), dtype=np.float32)
    for core in range(NCORES):
        out[core // HPC] += res.results[core]["out"]
    out += b_proj[None, None, :]
    return out, res


def kernel(**inputs):
    out, _ = _run(inputs)
    return out


# revision 28
# speedup vs baseline: 1.0173x; 1.0173x over previous
"""Causal self-attention (B=2, T=2048, E=1024, H=16, D=64) on 8 TRN2 cores.

Sharding: core = (batch, head-group): b = core // 4, heads 4g..4g+3 with
g = core % 4 (data parallel over batch x tensor parallel over heads).
Each core computes qkv projection for its 4 heads, causal attention, and
a partial output projection (its head rows of w_proj). Host sums the 4
partials per batch and adds b_proj.

All device compute is bf16 into fp32 PSUM (tolerance is 2e-2; bf16
matmuls run 1 col/cycle vs ~3 cycles/col for fp32 on TRN2's PE).

Device layout (per core):
  inputs (bf16): xt [1024, 2048] = x[b].T
                 wqk [1024, 512] = [w_q cols | w_k cols] for the 4 heads
                 wv  [1024, 256]
                 wp  [256, 1024] = w_proj rows for the 4 heads
                 masks [128, 4, 512] causal masks (diagonal positions)
  output (f32):  out [2048, 1024] partial projection

Matmul chain (no transposes needed):
  QK^T [512, 2048] = (x @ wqk)^T   via lhsT=wqk chunk, rhs=xt chunk
  V    [2048, 256+ones]            via lhsT=xt chunk, rhs=wv chunk
  S^T  [k,q] tiles                 via lhsT=K^T slice, rhs=Q^T slice
  O^T+sums = [V|1]^T P             via lhsT=v tile, rhs=exp(S^T) tile
  out  [2048,1024]                 via lhsT=attn^T chunk, rhs=wp chunk
O^T is written over the dead Q^T rows of the QK buffer.

Scheduling: attention head-pairs run single-stream with lag-1 PV (PV of
k-tile i-1 issues while exp(i) runs on ACT), and pure-PE "filler" work
(v/qk projection groups, out-proj tiles) is interleaved into the
attention loops to keep the PE busy during exp waits. Fillers are placed
so no filler produces data an earlier-emitted instruction consumes.
"""

import sys

sys.path.insert(0, "/opt/trn_rl_repo")

import numpy as np
import ml_dtypes

import concourse.bacc as bacc
import concourse.tile as tile
from concourse import mybir
from concourse.bass_utils import run_bass_kernel_spmd

F32 = mybir.dt.float32
BF16 = mybir.dt.bfloat16
Exp = mybir.ActivationFunctionType.Exp

B, T, E = 2, 2048, 1024
H, D = 16, 64
NCORES = 8
HPC = 4          # heads per core
QC = HPC * D     # 256 q cols per core
P = 128

_PROG = None


def _build():
    nc = bacc.Bacc("TRN2", target_bir_lowering=False, debug=False)

    xt_d = nc.dram_tensor("xt", [E, T], BF16, kind="ExternalInput")
    wqk_d = nc.dram_tensor("wqk", [E, 2 * QC], BF16, kind="ExternalInput")
    wv_d = nc.dram_tensor("wv", [E, QC], BF16, kind="ExternalInput")
    wp_d = nc.dram_tensor("wp", [QC, E], BF16, kind="ExternalInput")
    mk_d = nc.dram_tensor("masks", [P, 4, 512], BF16, kind="ExternalInput")
    out_d = nc.dram_tensor("out", [T, E], F32, kind="ExternalOutput")

    KC = E // P       # 8 contraction chunks over E
    NT = T // P       # 16 T tiles of 128

    with tile.TileContext(nc) as tc:
        with (
            tc.tile_pool(name="persist", bufs=1) as persist,
            tc.tile_pool(name="inp", bufs=1) as inp,
            tc.tile_pool(name="pt", bufs=4) as ptp,
            tc.tile_pool(name="small", bufs=2) as small,
            tc.tile_pool(name="denp", bufs=2) as denp,
            tc.tile_pool(name="stage", bufs=3) as stg,
            tc.tile_pool(name="big", bufs=2, space="PSUM") as big,
            tc.tile_pool(name="po", bufs=1, space="PSUM") as pop,
        ):
            # ---- persistent sbuf ----
            qk_sb = [persist.tile([P, T], BF16, name=f"qk{m}") for m in range(4)]
            at_sb = [persist.tile([P, T], BF16, name=f"at{c}") for c in range(2)]
            v_sb = [persist.tile([P, HPC, D + 1], BF16, name=f"v{t}") for t in range(NT)]
            mask_sb = persist.tile([P, 4, 512], BF16, name="masks")
            wp_sb = [persist.tile([P, E], BF16, name=f"wp{c}") for c in range(2)]

            # ---- input DMAs (first-needed first) ----
            xt_sb = [inp.tile([P, T], BF16, name=f"xt{c}") for c in range(KC)]
            wqk_sb = [inp.tile([P, 2 * QC], BF16, name=f"wqk{c}") for c in range(KC)]
            wv_sb = [inp.tile([P, QC], BF16, name=f"wv{c}") for c in range(KC)]
            # weights issue from the scalar/gpsimd queues, xt from sync —
            # DMA issue is ~0.6us per descriptor per queue, so parallel
            # queues cut the lead-in roughly in half
            for c in range(KC):
                nc.scalar.dma_start(out=wv_sb[c], in_=wv_d[c * P : (c + 1) * P, :])
                nc.sync.dma_start(
                    out=xt_sb[c][:, 0:1024], in_=xt_d[c * P : (c + 1) * P, 0:1024]
                )
                nc.gpsimd.dma_start(out=wqk_sb[c], in_=wqk_d[c * P : (c + 1) * P, :])
            nc.gpsimd.dma_start(out=mask_sb, in_=mk_d[:])
            for c in range(KC):
                nc.sync.dma_start(
                    out=xt_sb[c][:, 1024:2048],
                    in_=xt_d[c * P : (c + 1) * P, 1024:2048],
                )
            for c in range(2):
                nc.scalar.dma_start(out=wp_sb[c], in_=wp_d[c * P : (c + 1) * P, :])

            ones4 = persist.tile([P, HPC, 1], F32, name="ones4")
            nc.vector.memset(ones4, 1.0)
            for t in range(NT):
                nc.vector.tensor_copy(v_sb[t][:, :, D : D + 1], ones4)

            def v_group(t, eng):
                ps = big.tile([P, 2, 512], F32, name="ps")
                for c in range(KC):
                    nc.tensor.matmul(
                        ps[:, 0, :QC],
                        lhsT=xt_sb[c][:, t * P : (t + 1) * P],
                        rhs=wv_sb[c],
                        start=(c == 0),
                        stop=(c == KC - 1),
                    )
                cp = nc.scalar.copy if eng == "s" else nc.vector.tensor_copy
                cp(
                    v_sb[t][:, :, 0:D],
                    ps[:, 0, :QC].rearrange("p (h d) -> p h d", h=HPC),
                )

            def qk_group(m, u, eng):
                # c outer / nl inner: consecutive matmuls share lhsT
                ps = big.tile([P, 2, 512], F32, name="ps")
                for c in range(KC):
                    for nl in range(2):
                        nc.tensor.matmul(
                            ps[:, nl, :],
                            lhsT=wqk_sb[c][:, m * P : (m + 1) * P],
                            rhs=xt_sb[c][:, (2 * u + nl) * 512 : (2 * u + nl + 1) * 512],
                            start=(c == 0),
                            stop=(c == KC - 1),
                        )
                cp = nc.scalar.copy if eng == "s" else nc.vector.tensor_copy
                cp(
                    qk_sb[m][:, u * 1024 : (u + 1) * 1024],
                    ps.rearrange("p a b -> p (a b)"),
                )

            def out_tile(t, eng):
                ps = big.tile([P, 2, 512], F32, name="ps")
                for c in range(2):
                    for nl in range(2):
                        nc.tensor.matmul(
                            ps[:, nl, :],
                            lhsT=at_sb[c][:, t * P : (t + 1) * P],
                            rhs=wp_sb[c][:, nl * 512 : (nl + 1) * 512],
                            start=(c == 0),
                            stop=(c == 1),
                        )
                st = stg.tile([P, 1024], F32, name="st")
                cp = nc.scalar.copy if eng == "s" else nc.vector.tensor_copy
                cp(st, ps.rearrange("p a b -> p (a b)"))
                nc.sync.dma_start(out=out_d[t * P : (t + 1) * P, :], in_=st)

            def attn_unit(po, p, fillers=()):
                """Attention for heads (2po, 2po+1), query block p.

                The two heads live on partitions 0-63 / 64-127 of qk_sb[po]
                (Q^T) and qk_sb[2+po] (K^T). Their S^T matmuls have K=64
                contraction, so emitting them back-to-back packs them into
                disjoint row-strips of the PE array (row_grp auto-derived
                from the lhsT base partition) and they run concurrently.
                Per step (k-tile i, 512-col q-span jj) the S^T pair lands in
                one [128, 2, 512] PSUM tile (head per bank), one ACT exp
                covers both banks, then per-head PV accumulates. PV is
                emitted with lag 1 so the PE consumes step s-1 while ACT
                runs exp(s). `fillers` = (slot, closure) PE-only work.
                """
                Qh = qk_sb[po]
                Kh = qk_sb[2 + po]
                qbase = p * 1024
                psot = pop.tile([D + 1, 2, 2, 512], F32, name="pso")
                pso = [psot[:, 0], psot[:, 1]]
                nk = 8 * p + 8
                fills = {s: f for s, f in fillers}
                # steps: (i, jj, w) with w = causal trim start within the span
                steps = []
                for i in range(nk):
                    for jj in range(2):
                        m = i - 8 * p - 4 * jj
                        if m > 3:
                            continue
                        steps.append((i, jj, max(0, 128 * m), m))
                jlast = {jj: max(i for i, j, _, _ in steps if j == jj) for jj in range(2)}

                def emit_pv(i, jj, w, pt):
                    for hh in range(2):
                        nc.tensor.matmul(
                            pso[hh][:, jj, w:512],
                            lhsT=v_sb[i][:, 2 * po + hh, :],
                            rhs=pt[:, hh, w:512],
                            start=(i == 0),
                            stop=(i == jlast[jj]),
                        )

                pend = None
                for s, (i, jj, w, m) in enumerate(steps):
                    ps = big.tile([P, 2, 512], F32, name="ps")
                    pt = ptp.tile([P, 2, 512], BF16, name="pt")
                    q0 = qbase + jj * 512
                    # packed S^T pair: row strips 0-63 / 64-127 run concurrently
                    for hh in range(2):
                        nc.tensor.matmul(
                            ps[:, hh, w:512],
                            lhsT=Kh[64 * hh : 64 * hh + 64, i * P : (i + 1) * P],
                            rhs=Qh[64 * hh : 64 * hh + 64, q0 + w : q0 + 512],
                            start=True,
                            stop=True,
                        )
                    # filler before the lagged PV: keeps the PE stream moving
                    # while pso (at unit entry) or exp results are pending
                    f = fills.get(s)
                    if f is not None:
                        f()
                    # lagged PV: PE consumes pt(s-1) while ACT runs exp(s)
                    if pend is not None:
                        emit_pv(*pend)
                    # one exp over both heads' banks
                    nc.scalar.activation(pt[:, :, w:512], ps[:, :, w:512], Exp, scale=0.125)
                    # causal mask on diagonal tiles
                    if m >= 0:
                        for hh in range(2):
                            nc.vector.tensor_mul(
                                pt[:, hh, w:512],
                                pt[:, hh, w:512],
                                mask_sb[:, m, w:512],
                            )
                    pend = (i, jj, w, pt)
                emit_pv(*pend)
                # normalize and write O^T to the attn buffer; jj-major so
                # the jj=0 span's writes land first (out tiles of that span
                # unblock before the jj=1 chains finish)
                dens = []
                for hh in range(2):
                    den = denp.tile([1, 2, 512], F32, name="den")
                    nc.scalar.copy(den, pso[hh][D : D + 1, :, :])
                    dens.append(den)
                for jj in range(2):
                    for hh in range(2):
                        rec = small.tile([1, 512], F32, name="rec")
                        rb = small.tile([64, 512], F32, name="rb")
                        nc.vector.reciprocal_approx_fast(out=rec, in_=dens[hh][:, jj, :])
                        nc.gpsimd.partition_broadcast(rb, rec)
                        nc.vector.tensor_mul(
                            at_sb[po][
                                64 * hh : 64 * hh + 64,
                                qbase + jj * 512 : qbase + (jj + 1) * 512,
                            ],
                            pso[hh][0:D, jj, :],
                            rb,
                        )

            # ---- emission: lead-in projections, then attention with fillers
            for t in range(8):
                v_group(t, "s")
            qk_group(0, 0, "s")
            qk_group(2, 0, "s")
            # U1 = heads 0,1 @ q 0..1023 (12 steps)
            attn_unit(
                0, 0,
                [(0, lambda: qk_group(1, 0, "v")), (1, lambda: qk_group(3, 0, "v"))]
                + [(6 + j, lambda t=t: v_group(t, "v")) for j, t in enumerate(range(8, 12))],
            )
            # U2 = heads 2,3 @ q 0..1023 (12 steps)
            attn_unit(
                1, 0,
                [(0, lambda: qk_group(0, 1, "v")), (1, lambda: qk_group(2, 1, "v"))]
                + [(5 + j, lambda t=t: v_group(t, "v")) for j, t in enumerate(range(12, 16))],
            )
            # U3 = heads 0,1 @ q 1024..2047 (28 steps)
            attn_unit(
                0, 1,
                [(0, lambda: qk_group(1, 1, "v")), (1, lambda: qk_group(3, 1, "v"))]
                + [(8 + 4 * j, lambda t=t: out_tile(t, "v")) for j, t in enumerate(range(0, 4))],
            )
            # U4 = heads 2,3 @ q 1024..2047 (28 steps)
            attn_unit(
                1, 1,
                [(0, lambda: out_tile(4, "v")), (1, lambda: out_tile(5, "v")),
                 (12, lambda: out_tile(6, "v")), (22, lambda: out_tile(7, "v"))],
            )
            for t in range(8, NT):
                out_tile(t, "s" if t % 2 else "v")

    nc.compile()
    return nc


def _get_prog():
    global _PROG
    if _PROG is None:
        _PROG = _build()
    return _PROG


def _masks_np():
    kk = np.arange(P)[:, None]
    qq = np.arange(512)[None, :]
    return np.stack(
        [((128 * m + kk) <= qq) for m in range(4)], axis=1
    ).astype(ml_dtypes.bfloat16)


def _bf(a):
    return np.ascontiguousarray(a).astype(ml_dtypes.bfloat16)


def _shard(x, w_qkv, w_proj):
    masks = _masks_np()
    in_maps = []
    for core in range(NCORES):
        b, g = core // HPC, core % HPC
        c0 = g * QC
        in_maps.append(
            {
                "xt": _bf(x[b].T),
                "wqk": _bf(
                    np.concatenate(
                        [w_qkv[:, c0 : c0 + QC], w_qkv[:, E + c0 : E + c0 + QC]],
                        axis=1,
                    )
                ),
                "wv": _bf(w_qkv[:, 2 * E + c0 : 2 * E + c0 + QC]),
                "wp": _bf(w_proj[c0 : c0 + QC, :]),
                "masks": masks,
            }
        )
    return in_maps


def _run(inputs, **kwargs):
    x = np.asarray(inputs["x"], dtype=np.float32)
    w_qkv = np.asarray(inputs["w_qkv"], dtype=np.float32)
    w_proj = np.asarray(inputs["w_proj"], dtype=np.float32)
    b_proj = np.asarray(inputs["b_proj"], dtype=np.float32)

    nc = _get_prog()
    in_maps = _shard(x, w_qkv, w_proj)
    res = run_bass_kernel_spmd(nc, in_maps, core_ids=list(range(NCORES)), **kwargs)

    out = np.zeros((B, T, E), dtype=np.float32)
    for core in range(NCORES):
        out[core // HPC] += res.results[core]["out"]
    out += b_proj[None, None, :]
    return out, res


def kernel(**inputs):
    out, _ = _run(inputs)
    return out


# revision 29
# speedup vs baseline: 1.0212x; 1.0039x over previous
"""Causal self-attention (B=2, T=2048, E=1024, H=16, D=64) on 8 TRN2 cores.

Sharding: core = (batch, head-group): b = core // 4, heads 4g..4g+3 with
g = core % 4 (data parallel over batch x tensor parallel over heads).
Each core computes qkv projection for its 4 heads, causal attention, and
a partial output projection (its head rows of w_proj). Host sums the 4
partials per batch and adds b_proj.

All device compute is bf16 into fp32 PSUM (tolerance is 2e-2; bf16
matmuls run 1 col/cycle vs ~3 cycles/col for fp32 on TRN2's PE).

Device layout (per core):
  inputs (bf16): xt [1024, 2048] = x[b].T
                 wqk [1024, 512] = [w_q cols | w_k cols] for the 4 heads
                 wv  [1024, 256]
                 wp  [256, 1024] = w_proj rows for the 4 heads
                 masks [128, 4, 512] causal masks (diagonal positions)
  output (f32):  out [2048, 1024] partial projection

Matmul chain (no transposes needed):
  QK^T [512, 2048] = (x @ wqk)^T   via lhsT=wqk chunk, rhs=xt chunk
  V    [2048, 256+ones]            via lhsT=xt chunk, rhs=wv chunk
  S^T  [k,q] tiles                 via lhsT=K^T slice, rhs=Q^T slice
  O^T+sums = [V|1]^T P             via lhsT=v tile, rhs=exp(S^T) tile
  out  [2048,1024]                 via lhsT=attn^T chunk, rhs=wp chunk
O^T is written over the dead Q^T rows of the QK buffer.

Scheduling: attention head-pairs run single-stream with lag-1 PV (PV of
k-tile i-1 issues while exp(i) runs on ACT), and pure-PE "filler" work
(v/qk projection groups, out-proj tiles) is interleaved into the
attention loops to keep the PE busy during exp waits. Fillers are placed
so no filler produces data an earlier-emitted instruction consumes.
"""

import sys

sys.path.insert(0, "/opt/trn_rl_repo")

import numpy as np
import ml_dtypes

import concourse.bacc as bacc
import concourse.tile as tile
from concourse import mybir
from concourse.bass_utils import run_bass_kernel_spmd

F32 = mybir.dt.float32
BF16 = mybir.dt.bfloat16
Exp = mybir.ActivationFunctionType.Exp

B, T, E = 2, 2048, 1024
H, D = 16, 64
NCORES = 8
HPC = 4          # heads per core
QC = HPC * D     # 256 q cols per core
P = 128

_PROG = None


def _build():
    nc = bacc.Bacc("TRN2", target_bir_lowering=False, debug=False)

    xt_d = nc.dram_tensor("xt", [E, T], BF16, kind="ExternalInput")
    wqk_d = nc.dram_tensor("wqk", [E, 2 * QC], BF16, kind="ExternalInput")
    wv_d = nc.dram_tensor("wv", [E, QC], BF16, kind="ExternalInput")
    wp_d = nc.dram_tensor("wp", [QC, E], BF16, kind="ExternalInput")
    mk_d = nc.dram_tensor("masks", [P, 4, 512], BF16, kind="ExternalInput")
    out_d = nc.dram_tensor("out", [T, E], F32, kind="ExternalOutput")

    KC = E // P       # 8 contraction chunks over E
    NT = T // P       # 16 T tiles of 128

    with tile.TileContext(nc) as tc:
        with (
            tc.tile_pool(name="persist", bufs=1) as persist,
            tc.tile_pool(name="inp", bufs=1) as inp,
            tc.tile_pool(name="pt", bufs=4) as ptp,
            tc.tile_pool(name="small", bufs=2) as small,
            tc.tile_pool(name="denp", bufs=2) as denp,
            tc.tile_pool(name="stage", bufs=3) as stg,
            tc.tile_pool(name="big", bufs=2, space="PSUM") as big,
            tc.tile_pool(name="po", bufs=1, space="PSUM") as pop,
        ):
            # ---- persistent sbuf ----
            qk_sb = [persist.tile([P, T], BF16, name=f"qk{m}") for m in range(4)]
            at_sb = [persist.tile([P, T], BF16, name=f"at{c}") for c in range(2)]
            v_sb = [persist.tile([P, HPC, D + 1], BF16, name=f"v{t}") for t in range(NT)]
            mask_sb = persist.tile([P, 4, 512], BF16, name="masks")
            wp_sb = [persist.tile([P, E], BF16, name=f"wp{c}") for c in range(2)]

            # ---- input DMAs (first-needed first) ----
            xt_sb = [inp.tile([P, T], BF16, name=f"xt{c}") for c in range(KC)]
            wqk_sb = [inp.tile([P, 2 * QC], BF16, name=f"wqk{c}") for c in range(KC)]
            wv_sb = [inp.tile([P, QC], BF16, name=f"wv{c}") for c in range(KC)]
            for c in range(KC):
                nc.sync.dma_start(out=wv_sb[c], in_=wv_d[c * P : (c + 1) * P, :])
                nc.sync.dma_start(
                    out=xt_sb[c][:, 0:1024], in_=xt_d[c * P : (c + 1) * P, 0:1024]
                )
            for c in range(KC):
                nc.sync.dma_start(out=wqk_sb[c], in_=wqk_d[c * P : (c + 1) * P, :])
            nc.sync.dma_start(out=mask_sb, in_=mk_d[:])
            for c in range(KC):
                nc.sync.dma_start(
                    out=xt_sb[c][:, 1024:2048],
                    in_=xt_d[c * P : (c + 1) * P, 1024:2048],
                )
            for c in range(2):
                nc.sync.dma_start(out=wp_sb[c], in_=wp_d[c * P : (c + 1) * P, :])

            ones4 = persist.tile([P, HPC, 1], F32, name="ones4")
            nc.vector.memset(ones4, 1.0)
            for t in range(NT):
                nc.vector.tensor_copy(v_sb[t][:, :, D : D + 1], ones4)

            def v_group(t, eng):
                ps = big.tile([P, 2, 512], F32, name="ps")
                for c in range(KC):
                    nc.tensor.matmul(
                        ps[:, 0, :QC],
                        lhsT=xt_sb[c][:, t * P : (t + 1) * P],
                        rhs=wv_sb[c],
                        start=(c == 0),
                        stop=(c == KC - 1),
                    )
                cp = nc.scalar.copy if eng == "s" else nc.vector.tensor_copy
                cp(
                    v_sb[t][:, :, 0:D],
                    ps[:, 0, :QC].rearrange("p (h d) -> p h d", h=HPC),
                )

            def qk_group(m, u, eng):
                # c outer / nl inner: consecutive matmuls share lhsT
                ps = big.tile([P, 2, 512], F32, name="ps")
                for c in range(KC):
                    for nl in range(2):
                        nc.tensor.matmul(
                            ps[:, nl, :],
                            lhsT=wqk_sb[c][:, m * P : (m + 1) * P],
                            rhs=xt_sb[c][:, (2 * u + nl) * 512 : (2 * u + nl + 1) * 512],
                            start=(c == 0),
                            stop=(c == KC - 1),
                        )
                cp = nc.scalar.copy if eng == "s" else nc.vector.tensor_copy
                cp(
                    qk_sb[m][:, u * 1024 : (u + 1) * 1024],
                    ps.rearrange("p a b -> p (a b)"),
                )

            def out_tile(t, eng):
                ps = big.tile([P, 2, 512], F32, name="ps")
                for c in range(2):
                    for nl in range(2):
                        nc.tensor.matmul(
                            ps[:, nl, :],
                            lhsT=at_sb[c][:, t * P : (t + 1) * P],
                            rhs=wp_sb[c][:, nl * 512 : (nl + 1) * 512],
                            start=(c == 0),
                            stop=(c == 1),
                        )
                st = stg.tile([P, 1024], F32, name="st")
                cp = nc.scalar.copy if eng == "s" else nc.vector.tensor_copy
                cp(st, ps.rearrange("p a b -> p (a b)"))
                nc.sync.dma_start(out=out_d[t * P : (t + 1) * P, :], in_=st)

            def attn_unit(po, p, fillers=()):
                """Attention for heads (2po, 2po+1), query block p.

                The two heads live on partitions 0-63 / 64-127 of qk_sb[po]
                (Q^T) and qk_sb[2+po] (K^T). Their S^T matmuls have K=64
                contraction, so emitting them back-to-back packs them into
                disjoint row-strips of the PE array (row_grp auto-derived
                from the lhsT base partition) and they run concurrently.
                Per step (k-tile i, 512-col q-span jj) the S^T pair lands in
                one [128, 2, 512] PSUM tile (head per bank), one ACT exp
                covers both banks, then per-head PV accumulates. PV is
                emitted with lag 1 so the PE consumes step s-1 while ACT
                runs exp(s). `fillers` = (slot, closure) PE-only work.
                """
                Qh = qk_sb[po]
                Kh = qk_sb[2 + po]
                qbase = p * 1024
                psot = pop.tile([D + 1, 2, 2, 512], F32, name="pso")
                pso = [psot[:, 0], psot[:, 1]]
                nk = 8 * p + 8
                fills = {s: f for s, f in fillers}
                # steps: (i, jj, w) with w = causal trim start within the span.
                # For p=1, walk k-tiles interleaving the narrow diagonal tiles
                # (i>=8, exp-overhead-dominated, PE-starved) among the
                # full-width ones so neither engine starves in a run of
                # same-shaped steps. PSUM accumulation order is commutative.
                iorder = (
                    [0, 12, 1, 13, 2, 14, 3, 15, 4, 8, 5, 9, 6, 10, 7, 11]
                    if p == 1
                    else range(nk)
                )
                steps = []
                for i in iorder:
                    for jj in range(2):
                        m = i - 8 * p - 4 * jj
                        if m > 3:
                            continue
                        steps.append((i, jj, max(0, 128 * m), m))
                jfirst = {}
                jlast = {}
                for i, j, _, _ in steps:
                    jfirst.setdefault(j, i)
                    jlast[j] = i

                def emit_pv(i, jj, w, pt):
                    for hh in range(2):
                        nc.tensor.matmul(
                            pso[hh][:, jj, w:512],
                            lhsT=v_sb[i][:, 2 * po + hh, :],
                            rhs=pt[:, hh, w:512],
                            start=(i == jfirst[jj]),
                            stop=(i == jlast[jj]),
                        )

                pend = None
                for s, (i, jj, w, m) in enumerate(steps):
                    ps = big.tile([P, 2, 512], F32, name="ps")
                    pt = ptp.tile([P, 2, 512], BF16, name="pt")
                    q0 = qbase + jj * 512
                    # packed S^T pair: row strips 0-63 / 64-127 run concurrently
                    for hh in range(2):
                        nc.tensor.matmul(
                            ps[:, hh, w:512],
                            lhsT=Kh[64 * hh : 64 * hh + 64, i * P : (i + 1) * P],
                            rhs=Qh[64 * hh : 64 * hh + 64, q0 + w : q0 + 512],
                            start=True,
                            stop=True,
                        )
                    # filler before the lagged PV: keeps the PE stream moving
                    # while pso (at unit entry) or exp results are pending
                    f = fills.get(s)
                    if f is not None:
                        f()
                    # lagged PV: PE consumes pt(s-1) while ACT runs exp(s)
                    if pend is not None:
                        emit_pv(*pend)
                    # one exp over both heads' banks
                    nc.scalar.activation(pt[:, :, w:512], ps[:, :, w:512], Exp, scale=0.125)
                    # causal mask on diagonal tiles
                    if m >= 0:
                        for hh in range(2):
                            nc.vector.tensor_mul(
                                pt[:, hh, w:512],
                                pt[:, hh, w:512],
                                mask_sb[:, m, w:512],
                            )
                    pend = (i, jj, w, pt)
                emit_pv(*pend)
                # normalize and write O^T to the attn buffer; jj-major so
                # the jj=0 span's writes land first (out tiles of that span
                # unblock before the jj=1 chains finish)
                dens = []
                for hh in range(2):
                    den = denp.tile([1, 2, 512], F32, name="den")
                    nc.scalar.copy(den, pso[hh][D : D + 1, :, :])
                    dens.append(den)
                for jj in range(2):
                    for hh in range(2):
                        rec = small.tile([1, 512], F32, name="rec")
                        rb = small.tile([64, 512], F32, name="rb")
                        nc.vector.reciprocal_approx_fast(out=rec, in_=dens[hh][:, jj, :])
                        nc.gpsimd.partition_broadcast(rb, rec)
                        nc.vector.tensor_mul(
                            at_sb[po][
                                64 * hh : 64 * hh + 64,
                                qbase + jj * 512 : qbase + (jj + 1) * 512,
                            ],
                            pso[hh][0:D, jj, :],
                            rb,
                        )

            # ---- emission: lead-in projections, then attention with fillers
            for t in range(8):
                v_group(t, "s")
            qk_group(0, 0, "s")
            qk_group(2, 0, "s")
            # U1 = heads 0,1 @ q 0..1023 (12 steps)
            attn_unit(
                0, 0,
                [(0, lambda: qk_group(1, 0, "v")), (1, lambda: qk_group(3, 0, "v"))]
                + [(6 + j, lambda t=t: v_group(t, "v")) for j, t in enumerate(range(8, 12))],
            )
            # U2 = heads 2,3 @ q 0..1023 (12 steps)
            attn_unit(
                1, 0,
                [(0, lambda: qk_group(0, 1, "v")), (1, lambda: qk_group(2, 1, "v"))]
                + [(5 + j, lambda t=t: v_group(t, "v")) for j, t in enumerate(range(12, 16))],
            )
            # U3 = heads 0,1 @ q 1024..2047 (28 steps)
            attn_unit(
                0, 1,
                [(0, lambda: qk_group(1, 1, "v")), (1, lambda: qk_group(3, 1, "v"))]
                + [(8 + 4 * j, lambda t=t: out_tile(t, "v")) for j, t in enumerate(range(0, 4))],
            )
            # U4 = heads 2,3 @ q 1024..2047 (28 steps)
            attn_unit(
                1, 1,
                [(0, lambda: out_tile(4, "v")), (1, lambda: out_tile(5, "v")),
                 (12, lambda: out_tile(6, "v")), (22, lambda: out_tile(7, "v"))],
            )
            for t in range(8, NT):
                out_tile(t, "s" if t % 2 else "v")

    nc.compile()
    return nc


def _get_prog():
    global _PROG
    if _PROG is None:
        _PROG = _build()
    return _PROG


def _masks_np():
    kk = np.arange(P)[:, None]
    qq = np.arange(512)[None, :]
    return np.stack(
        [((128 * m + kk) <= qq) for m in range(4)], axis=1
    ).astype(ml_dtypes.bfloat16)


def _bf(a):
    return np.ascontiguousarray(a).astype(ml_dtypes.bfloat16)


def _shard(x, w_qkv, w_proj):
    masks = _masks_np()
    in_maps = []
    for core in range(NCORES):
        b, g = core // HPC, core % HPC
        c0 = g * QC
        in_maps.append(
            {
                "xt": _bf(x[b].T),
                "wqk": _bf(
                    np.concatenate(
                        [w_qkv[:, c0 : c0 + QC], w_qkv[:, E + c0 : E + c0 + QC]],
                        axis=1,
                    )
                ),
                "wv": _bf(w_qkv[:, 2 * E + c0 : 2 * E + c0 + QC]),
                "wp": _bf(w_proj[c0 : c0 + QC, :]),
                "masks": masks,
            }
        )
    return in_maps


def _run(inputs, **kwargs):
    x = np.asarray(inputs["x"], dtype=np.float32)
    w_qkv = np.asarray(inputs["w_qkv"], dtype=np.float32)
    w_proj = np.asarray(inputs["w_proj"], dtype=np.float32)
    b_proj = np.asarray(inputs["b_proj"], dtype=np.float32)

    nc = _get_prog()
    in_maps = _shard(x, w_qkv, w_proj)
    res = run_bass_kernel_spmd(nc, in_maps, core_ids=list(range(NCORES)), **kwargs)

    out = np.zeros((B, T, E), dtype=np.float32)
    for core in range(NCORES):
        out[core // HPC] += res.results[core]["out"]
    out += b_proj[None, None, :]
    return out, res


def kernel(**inputs):
    out, _ = _run(inputs)
    return out


# revision 30
# speedup vs baseline: 1.0472x; 1.0254x over previous
"""Causal self-attention (B=2, T=2048, E=1024, H=16, D=64) on 8 TRN2 cores.

Sharding: core = (batch, head-group): b = core // 4, heads 4g..4g+3 with
g = core % 4 (data parallel over batch x tensor parallel over heads).
Each core computes qkv projection for its 4 heads, causal attention, and
a partial output projection (its head rows of w_proj). Host sums the 4
partials per batch and adds b_proj.

All device compute is bf16 into fp32 PSUM (tolerance is 2e-2; bf16
matmuls run 1 col/cycle vs ~3 cycles/col for fp32 on TRN2's PE).

Device layout (per core):
  inputs (bf16): xt [1024, 2048] = x[b].T
                 wqk [1024, 512] = [w_q cols | w_k cols] for the 4 heads
                 wv  [1024, 256]
                 wp  [256, 1024] = w_proj rows for the 4 heads
                 masks [128, 4, 512] causal masks (diagonal positions)
  output (f32):  out [2048, 1024] partial projection

Matmul chain (no transposes needed):
  QK^T [512, 2048] = (x @ wqk)^T   via lhsT=wqk chunk, rhs=xt chunk
  V    [2048, 256+ones]            via lhsT=xt chunk, rhs=wv chunk
  S^T  [k,q] tiles                 via lhsT=K^T slice, rhs=Q^T slice
  O^T+sums = [V|1]^T P             via lhsT=v tile, rhs=exp(S^T) tile
  out  [2048,1024]                 via lhsT=attn^T chunk, rhs=wp chunk
O^T is written over the dead Q^T rows of the QK buffer.

Scheduling: attention head-pairs run single-stream with lag-1 PV (PV of
k-tile i-1 issues while exp(i) runs on ACT), and pure-PE "filler" work
(v/qk projection groups, out-proj tiles) is interleaved into the
attention loops to keep the PE busy during exp waits. Fillers are placed
so no filler produces data an earlier-emitted instruction consumes.
"""

import sys

sys.path.insert(0, "/opt/trn_rl_repo")

import numpy as np
import ml_dtypes

import concourse.bacc as bacc
import concourse.tile as tile
from concourse import mybir
from concourse.bass_utils import run_bass_kernel_spmd

F32 = mybir.dt.float32
BF16 = mybir.dt.bfloat16
Exp = mybir.ActivationFunctionType.Exp

B, T, E = 2, 2048, 1024
H, D = 16, 64
NCORES = 8
HPC = 4          # heads per core
QC = HPC * D     # 256 q cols per core
P = 128

_PROG = None


def _build():
    nc = bacc.Bacc("TRN2", target_bir_lowering=False, debug=False)

    xt_d = nc.dram_tensor("xt", [E, T], BF16, kind="ExternalInput")
    wqk_d = nc.dram_tensor("wqk", [E, 2 * QC], BF16, kind="ExternalInput")
    wv_d = nc.dram_tensor("wv", [E, QC], BF16, kind="ExternalInput")
    wp_d = nc.dram_tensor("wp", [QC, E], BF16, kind="ExternalInput")
    mk_d = nc.dram_tensor("masks", [P, 4, 512], BF16, kind="ExternalInput")
    out_d = nc.dram_tensor("out", [T, E], F32, kind="ExternalOutput")

    KC = E // P       # 8 contraction chunks over E
    NT = T // P       # 16 T tiles of 128

    with tile.TileContext(nc) as tc:
        with (
            tc.tile_pool(name="persist", bufs=1) as persist,
            tc.tile_pool(name="inp", bufs=1) as inp,
            tc.tile_pool(name="pt", bufs=4) as ptp,
            tc.tile_pool(name="small", bufs=2) as small,
            tc.tile_pool(name="denp", bufs=2) as denp,
            tc.tile_pool(name="stage", bufs=3) as stg,
            tc.tile_pool(name="big", bufs=2, space="PSUM") as big,
            tc.tile_pool(name="po", bufs=1, space="PSUM") as pop,
        ):
            # ---- persistent sbuf ----
            qk_sb = [persist.tile([P, T], BF16, name=f"qk{m}") for m in range(4)]
            at_sb = [persist.tile([P, T], BF16, name=f"at{c}") for c in range(2)]
            v_sb = [persist.tile([P, HPC, D + 1], BF16, name=f"v{t}") for t in range(NT)]
            mask_sb = persist.tile([P, 4, 512], BF16, name="masks")
            wp_sb = [persist.tile([P, E], BF16, name=f"wp{c}") for c in range(2)]

            # ---- input DMAs (first-needed first) ----
            xt_sb = [inp.tile([P, T], BF16, name=f"xt{c}") for c in range(KC)]
            wqk_sb = [inp.tile([P, 2 * QC], BF16, name=f"wqk{c}") for c in range(KC)]
            wv_sb = [inp.tile([P, QC], BF16, name=f"wv{c}") for c in range(KC)]
            for c in range(KC):
                nc.sync.dma_start(out=wv_sb[c], in_=wv_d[c * P : (c + 1) * P, :])
                nc.sync.dma_start(
                    out=xt_sb[c][:, 0:1024], in_=xt_d[c * P : (c + 1) * P, 0:1024]
                )
            for c in range(KC):
                nc.sync.dma_start(out=wqk_sb[c], in_=wqk_d[c * P : (c + 1) * P, :])
            nc.sync.dma_start(out=mask_sb, in_=mk_d[:])
            for c in range(KC):
                nc.sync.dma_start(
                    out=xt_sb[c][:, 1024:2048],
                    in_=xt_d[c * P : (c + 1) * P, 1024:2048],
                )
            for c in range(2):
                nc.sync.dma_start(out=wp_sb[c], in_=wp_d[c * P : (c + 1) * P, :])

            ones4 = persist.tile([P, HPC, 1], F32, name="ones4")
            nc.vector.memset(ones4, 1.0)
            for t in range(NT):
                nc.vector.tensor_copy(v_sb[t][:, :, D : D + 1], ones4)

            def v_group(t, eng):
                ps = big.tile([P, 2, 512], F32, name="ps")
                for c in range(KC):
                    nc.tensor.matmul(
                        ps[:, 0, :QC],
                        lhsT=xt_sb[c][:, t * P : (t + 1) * P],
                        rhs=wv_sb[c],
                        start=(c == 0),
                        stop=(c == KC - 1),
                    )
                cp = nc.scalar.copy if eng == "s" else nc.vector.tensor_copy
                cp(
                    v_sb[t][:, :, 0:D],
                    ps[:, 0, :QC].rearrange("p (h d) -> p h d", h=HPC),
                )

            def qk_group(m, u, eng):
                # c outer / nl inner: consecutive matmuls share lhsT
                ps = big.tile([P, 2, 512], F32, name="ps")
                for c in range(KC):
                    for nl in range(2):
                        nc.tensor.matmul(
                            ps[:, nl, :],
                            lhsT=wqk_sb[c][:, m * P : (m + 1) * P],
                            rhs=xt_sb[c][:, (2 * u + nl) * 512 : (2 * u + nl + 1) * 512],
                            start=(c == 0),
                            stop=(c == KC - 1),
                        )
                cp = nc.scalar.copy if eng == "s" else nc.vector.tensor_copy
                cp(
                    qk_sb[m][:, u * 1024 : (u + 1) * 1024],
                    ps.rearrange("p a b -> p (a b)"),
                )

            def out_tile(t, eng):
                ps = big.tile([P, 2, 512], F32, name="ps")
                for c in range(2):
                    for nl in range(2):
                        nc.tensor.matmul(
                            ps[:, nl, :],
                            lhsT=at_sb[c][:, t * P : (t + 1) * P],
                            rhs=wp_sb[c][:, nl * 512 : (nl + 1) * 512],
                            start=(c == 0),
                            stop=(c == 1),
                        )
                st = stg.tile([P, 1024], F32, name="st")
                cp = nc.scalar.copy if eng == "s" else nc.vector.tensor_copy
                cp(st, ps.rearrange("p a b -> p (a b)"))
                nc.sync.dma_start(out=out_d[t * P : (t + 1) * P, :], in_=st)

            def attn_unit(po, p, fillers=()):
                """Attention for heads (2po, 2po+1), query block p.

                The two heads live on partitions 0-63 / 64-127 of qk_sb[po]
                (Q^T) and qk_sb[2+po] (K^T). Their S^T matmuls have K=64
                contraction, so emitting them back-to-back packs them into
                disjoint row-strips of the PE array (row_grp auto-derived
                from the lhsT base partition) and they run concurrently.
                Per step (k-tile i, 512-col q-span jj) the S^T pair lands in
                one [128, 2, 512] PSUM tile (head per bank), one ACT exp
                covers both banks, then per-head PV accumulates. PV is
                emitted with lag 1 so the PE consumes step s-1 while ACT
                runs exp(s). `fillers` = (slot, closure) PE-only work.
                """
                Qh = qk_sb[po]
                Kh = qk_sb[2 + po]
                qbase = p * 1024
                psot = pop.tile([D + 1, 2, 2, 512], F32, name="pso")
                pso = [psot[:, 0], psot[:, 1]]
                nk = 8 * p + 8
                fills = {s: f for s, f in fillers}
                # steps: (i, jj, w) with w = causal trim start within the span
                steps = []
                for i in range(nk):
                    for jj in range(2):
                        m = i - 8 * p - 4 * jj
                        if m > 3:
                            continue
                        steps.append((i, jj, max(0, 128 * m), m))
                jlast = {jj: max(i for i, j, _, _ in steps if j == jj) for jj in range(2)}

                def emit_pv(i, jj, w, pt):
                    for hh in range(2):
                        nc.tensor.matmul(
                            pso[hh][:, jj, w:512],
                            lhsT=v_sb[i][:, 2 * po + hh, :],
                            rhs=pt[:, hh, w:512],
                            start=(i == 0),
                            stop=(i == jlast[jj]),
                        )

                pend = None
                for s, (i, jj, w, m) in enumerate(steps):
                    ps = big.tile([P, 2, 512], F32, name="ps")
                    pt = ptp.tile([P, 2, 512], BF16, name="pt")
                    q0 = qbase + jj * 512
                    # packed S^T pair: row strips 0-63 / 64-127 run concurrently
                    for hh in range(2):
                        nc.tensor.matmul(
                            ps[:, hh, w:512],
                            lhsT=Kh[64 * hh : 64 * hh + 64, i * P : (i + 1) * P],
                            rhs=Qh[64 * hh : 64 * hh + 64, q0 + w : q0 + 512],
                            start=True,
                            stop=True,
                        )
                    # filler before the lagged PV: keeps the PE stream moving
                    # while pso (at unit entry) or exp results are pending
                    f = fills.get(s)
                    if f is not None:
                        f()
                    # lagged PV: PE consumes pt(s-1) while ACT runs exp(s)
                    if pend is not None:
                        emit_pv(*pend)
                    # one exp over both heads' banks
                    nc.scalar.activation(pt[:, :, w:512], ps[:, :, w:512], Exp, scale=0.125)
                    # causal mask on diagonal tiles
                    if m >= 0:
                        for hh in range(2):
                            nc.vector.tensor_mul(
                                pt[:, hh, w:512],
                                pt[:, hh, w:512],
                                mask_sb[:, m, w:512],
                            )
                    pend = (i, jj, w, pt)
                emit_pv(*pend)
                # normalize and write O^T to the attn buffer; jj-major so
                # the jj=0 span's writes land first (out tiles of that span
                # unblock before the jj=1 chains finish)
                dens = []
                for hh in range(2):
                    den = denp.tile([1, 2, 512], F32, name="den")
                    nc.scalar.copy(den, pso[hh][D : D + 1, :, :])
                    dens.append(den)
                for jj in range(2):
                    for hh in range(2):
                        rec = small.tile([1, 512], F32, name="rec")
                        rb = small.tile([64, 512], F32, name="rb")
                        nc.vector.reciprocal_approx_fast(out=rec, in_=dens[hh][:, jj, :])
                        nc.gpsimd.partition_broadcast(rb, rec)
                        nc.vector.tensor_mul(
                            at_sb[po][
                                64 * hh : 64 * hh + 64,
                                qbase + jj * 512 : qbase + (jj + 1) * 512,
                            ],
                            pso[hh][0:D, jj, :],
                            rb,
                        )

            # ---- emission: lead-in projections, then attention with fillers
            for t in range(8):
                v_group(t, "s")
            qk_group(0, 0, "s")
            qk_group(2, 0, "s")
            # U1 = heads 0,1 @ q 0..1023 (12 steps)
            attn_unit(
                0, 0,
                [(0, lambda: qk_group(1, 0, "v")), (1, lambda: qk_group(3, 0, "v"))]
                + [(6 + j, lambda t=t: v_group(t, "v")) for j, t in enumerate(range(8, 12))],
            )
            # U2 = heads 2,3 @ q 0..1023 (12 steps)
            attn_unit(
                1, 0,
                [(0, lambda: qk_group(0, 1, "v")), (1, lambda: qk_group(2, 1, "v"))]
                + [(5 + j, lambda t=t: v_group(t, "v")) for j, t in enumerate(range(12, 16))],
            )
            # U3 = heads 0,1 @ q 1024..2047 (28 steps)
            attn_unit(
                0, 1,
                [(0, lambda: qk_group(1, 1, "v")), (1, lambda: qk_group(3, 1, "v"))]
                + [(8 + 4 * j, lambda t=t: out_tile(t, "v")) for j, t in enumerate(range(0, 4))],
            )
            # U4 = heads 2,3 @ q 1024..2047 (28 steps)
            attn_unit(
                1, 1,
                [(0, lambda: out_tile(4, "v")), (1, lambda: out_tile(5, "v")),
                 (12, lambda: out_tile(6, "v")), (22, lambda: out_tile(7, "v"))],
            )
            for t in range(8, NT):
                out_tile(t, "s" if t % 2 else "v")

    nc.compile()
    return nc


def _get_prog():
    global _PROG
    if _PROG is None:
        _PROG = _build()
    return _PROG


def _masks_np():
    kk = np.arange(P)[:, None]
    qq = np.arange(512)[None, :]
    return np.stack(
        [((128 * m + kk) <= qq) for m in range(4)], axis=1
    ).astype(ml_dtypes.bfloat16)


def _bf(a):
    return np.ascontiguousarray(a).astype(ml_dtypes.bfloat16)


def _shard(x, w_qkv, w_proj):
    masks = _masks_np()
    in_maps = []
    for core in range(NCORES):
        b, g = core // HPC, core % HPC
        c0 = g * QC
        in_maps.append(
            {
                "xt": _bf(x[b].T),
                "wqk": _bf(
                    np.concatenate(
                        [w_qkv[:, c0 : c0 + QC], w_qkv[:, E + c0 : E + c0 + QC]],
                        axis=1,
                    )
                ),
                "wv": _bf(w_qkv[:, 2 * E + c0 : 2 * E + c0 + QC]),
                "wp": _bf(w_proj[c0 : c0 + QC, :]),
                "masks": masks,
            }
        )
    return in_maps


def _run(inputs, **kwargs):
    x = np.asarray(inputs["x"], dtype=np.float32)
    w_qkv = np.asarray(inputs["w_qkv"], dtype=np.float32)
    w_proj = np.asarray(inputs["w_proj"], dtype=np.float32)
    b_proj = np.asarray(inputs["b_proj"], dtype=np.float32)

    nc = _get_prog()
    in_maps = _shard(x, w_qkv, w_proj)
    res = run_bass_kernel_spmd(nc, in_maps, core_ids=list(range(NCORES)), **kwargs)

    out = np.zeros((B, T, E), dtype=np.float32)
    for core in range(NCORES):
        out[core // HPC] += res.results[core]["out"]
    out += b_proj[None, None, :]
    return out, res


def kernel(**inputs):
    out, _ = _run(inputs)
    return out


# revision 31
# speedup vs baseline: 1.0506x; 1.0033x over previous
"""Causal self-attention (B=2, T=2048, E=1024, H=16, D=64) on 8 TRN2 cores.

Sharding: core = (batch, head-group): b = core // 4, heads 4g..4g+3 with
g = core % 4 (data parallel over batch x tensor parallel over heads).
Each core computes qkv projection for its 4 heads, causal attention, and
a partial output projection (its head rows of w_proj). Host sums the 4
partials per batch and adds b_proj.

All device compute is bf16 into fp32 PSUM (tolerance is 2e-2; bf16
matmuls run 1 col/cycle vs ~3 cycles/col for fp32 on TRN2's PE).

Device layout (per core):
  inputs (bf16): xt [1024, 2048] = x[b].T
                 wqk [1024, 512] = [w_q cols | w_k cols] for the 4 heads
                 wv  [1024, 256]
                 wp  [256, 1024] = w_proj rows for the 4 heads
                 masks [128, 4, 512] causal masks (diagonal positions)
  output (f32):  out [2048, 1024] partial projection

Matmul chain (no transposes needed):
  QK^T [512, 2048] = (x @ wqk)^T   via lhsT=wqk chunk, rhs=xt chunk
  V    [2048, 256+ones]            via lhsT=xt chunk, rhs=wv chunk
  S^T  [k,q] tiles                 via lhsT=K^T slice, rhs=Q^T slice
  O^T+sums = [V|1]^T P             via lhsT=v tile, rhs=exp(S^T) tile
  out  [2048,1024]                 via lhsT=attn^T chunk, rhs=wp chunk
O^T is written over the dead Q^T rows of the QK buffer.

Scheduling: attention head-pairs run single-stream with lag-1 PV (PV of
k-tile i-1 issues while exp(i) runs on ACT), and pure-PE "filler" work
(v/qk projection groups, out-proj tiles) is interleaved into the
attention loops to keep the PE busy during exp waits. Fillers are placed
so no filler produces data an earlier-emitted instruction consumes.
"""

import sys

sys.path.insert(0, "/opt/trn_rl_repo")

import numpy as np
import ml_dtypes

import concourse.bacc as bacc
import concourse.tile as tile
from concourse import mybir
from concourse.bass_utils import run_bass_kernel_spmd

F32 = mybir.dt.float32
BF16 = mybir.dt.bfloat16
Exp = mybir.ActivationFunctionType.Exp

B, T, E = 2, 2048, 1024
H, D = 16, 64
NCORES = 8
HPC = 4          # heads per core
QC = HPC * D     # 256 q cols per core
P = 128

_PROG = None


def _build():
    nc = bacc.Bacc("TRN2", target_bir_lowering=False, debug=False)

    xt_d = nc.dram_tensor("xt", [E, T], BF16, kind="ExternalInput")
    wqk_d = nc.dram_tensor("wqk", [E, 2 * QC], BF16, kind="ExternalInput")
    wv_d = nc.dram_tensor("wv", [E, QC], BF16, kind="ExternalInput")
    wp_d = nc.dram_tensor("wp", [QC, E], BF16, kind="ExternalInput")
    mk_d = nc.dram_tensor("masks", [P, 4, 512], BF16, kind="ExternalInput")
    out_d = nc.dram_tensor("out", [T, E], BF16, kind="ExternalOutput")

    KC = E // P       # 8 contraction chunks over E
    NT = T // P       # 16 T tiles of 128

    with tile.TileContext(nc) as tc:
        with (
            tc.tile_pool(name="persist", bufs=1) as persist,
            tc.tile_pool(name="inp", bufs=1) as inp,
            tc.tile_pool(name="pt", bufs=4) as ptp,
            tc.tile_pool(name="small", bufs=2) as small,
            tc.tile_pool(name="denp", bufs=2) as denp,
            tc.tile_pool(name="stage", bufs=3) as stg,
            tc.tile_pool(name="big", bufs=2, space="PSUM") as big,
            tc.tile_pool(name="po", bufs=1, space="PSUM") as pop,
        ):
            # ---- persistent sbuf ----
            qk_sb = [persist.tile([P, T], BF16, name=f"qk{m}") for m in range(4)]
            at_sb = [persist.tile([P, T], BF16, name=f"at{c}") for c in range(2)]
            v_sb = [persist.tile([P, HPC, D + 1], BF16, name=f"v{t}") for t in range(NT)]
            mask_sb = persist.tile([P, 4, 512], BF16, name="masks")
            wp_sb = [persist.tile([P, E], BF16, name=f"wp{c}") for c in range(2)]

            # ---- input DMAs (first-needed first) ----
            xt_sb = [inp.tile([P, T], BF16, name=f"xt{c}") for c in range(KC)]
            wqk_sb = [inp.tile([P, 2 * QC], BF16, name=f"wqk{c}") for c in range(KC)]
            wv_sb = [inp.tile([P, QC], BF16, name=f"wv{c}") for c in range(KC)]
            for c in range(KC):
                nc.sync.dma_start(out=wv_sb[c], in_=wv_d[c * P : (c + 1) * P, :])
                nc.sync.dma_start(
                    out=xt_sb[c][:, 0:1024], in_=xt_d[c * P : (c + 1) * P, 0:1024]
                )
            for c in range(KC):
                nc.sync.dma_start(out=wqk_sb[c], in_=wqk_d[c * P : (c + 1) * P, :])
            nc.sync.dma_start(out=mask_sb, in_=mk_d[:])
            for c in range(KC):
                nc.sync.dma_start(
                    out=xt_sb[c][:, 1024:2048],
                    in_=xt_d[c * P : (c + 1) * P, 1024:2048],
                )
            for c in range(2):
                nc.sync.dma_start(out=wp_sb[c], in_=wp_d[c * P : (c + 1) * P, :])

            ones4 = persist.tile([P, HPC, 1], F32, name="ones4")
            nc.vector.memset(ones4, 1.0)
            for t in range(NT):
                nc.vector.tensor_copy(v_sb[t][:, :, D : D + 1], ones4)

            def v_group(t, eng):
                ps = big.tile([P, 2, 512], F32, name="ps")
                for c in range(KC):
                    nc.tensor.matmul(
                        ps[:, 0, :QC],
                        lhsT=xt_sb[c][:, t * P : (t + 1) * P],
                        rhs=wv_sb[c],
                        start=(c == 0),
                        stop=(c == KC - 1),
                    )
                cp = nc.scalar.copy if eng == "s" else nc.vector.tensor_copy
                cp(
                    v_sb[t][:, :, 0:D],
                    ps[:, 0, :QC].rearrange("p (h d) -> p h d", h=HPC),
                )

            def qk_group(m, u, eng):
                # c outer / nl inner: consecutive matmuls share lhsT
                ps = big.tile([P, 2, 512], F32, name="ps")
                for c in range(KC):
                    for nl in range(2):
                        nc.tensor.matmul(
                            ps[:, nl, :],
                            lhsT=wqk_sb[c][:, m * P : (m + 1) * P],
                            rhs=xt_sb[c][:, (2 * u + nl) * 512 : (2 * u + nl + 1) * 512],
                            start=(c == 0),
                            stop=(c == KC - 1),
                        )
                cp = nc.scalar.copy if eng == "s" else nc.vector.tensor_copy
                cp(
                    qk_sb[m][:, u * 1024 : (u + 1) * 1024],
                    ps.rearrange("p a b -> p (a b)"),
                )

            def out_tile(t, eng):
                ps = big.tile([P, 2, 512], F32, name="ps")
                for c in range(2):
                    for nl in range(2):
                        nc.tensor.matmul(
                            ps[:, nl, :],
                            lhsT=at_sb[c][:, t * P : (t + 1) * P],
                            rhs=wp_sb[c][:, nl * 512 : (nl + 1) * 512],
                            start=(c == 0),
                            stop=(c == 1),
                        )
                st = stg.tile([P, 1024], BF16, name="st")
                cp = nc.scalar.copy if eng == "s" else nc.vector.tensor_copy
                cp(st, ps.rearrange("p a b -> p (a b)"))
                nc.sync.dma_start(out=out_d[t * P : (t + 1) * P, :], in_=st)

            def attn_unit(po, p, fillers=()):
                """Attention for heads (2po, 2po+1), query block p.

                The two heads live on partitions 0-63 / 64-127 of qk_sb[po]
                (Q^T) and qk_sb[2+po] (K^T). Their S^T matmuls have K=64
                contraction, so emitting them back-to-back packs them into
                disjoint row-strips of the PE array (row_grp auto-derived
                from the lhsT base partition) and they run concurrently.
                Per step (k-tile i, 512-col q-span jj) the S^T pair lands in
                one [128, 2, 512] PSUM tile (head per bank), one ACT exp
                covers both banks, then per-head PV accumulates. PV is
                emitted with lag 1 so the PE consumes step s-1 while ACT
                runs exp(s). `fillers` = (slot, closure) PE-only work.
                """
                Qh = qk_sb[po]
                Kh = qk_sb[2 + po]
                qbase = p * 1024
                psot = pop.tile([D + 1, 2, 2, 512], F32, name="pso")
                pso = [psot[:, 0], psot[:, 1]]
                nk = 8 * p + 8
                fills = {s: f for s, f in fillers}
                # steps: (i, jj, w) with w = causal trim start within the span
                steps = []
                for i in range(nk):
                    for jj in range(2):
                        m = i - 8 * p - 4 * jj
                        if m > 3:
                            continue
                        steps.append((i, jj, max(0, 128 * m), m))
                jlast = {jj: max(i for i, j, _, _ in steps if j == jj) for jj in range(2)}

                def emit_pv(i, jj, w, pt):
                    for hh in range(2):
                        nc.tensor.matmul(
                            pso[hh][:, jj, w:512],
                            lhsT=v_sb[i][:, 2 * po + hh, :],
                            rhs=pt[:, hh, w:512],
                            start=(i == 0),
                            stop=(i == jlast[jj]),
                        )

                pend = None
                for s, (i, jj, w, m) in enumerate(steps):
                    ps = big.tile([P, 2, 512], F32, name="ps")
                    pt = ptp.tile([P, 2, 512], BF16, name="pt")
                    q0 = qbase + jj * 512
                    # packed S^T pair: row strips 0-63 / 64-127 run concurrently
                    for hh in range(2):
                        nc.tensor.matmul(
                            ps[:, hh, w:512],
                            lhsT=Kh[64 * hh : 64 * hh + 64, i * P : (i + 1) * P],
                            rhs=Qh[64 * hh : 64 * hh + 64, q0 + w : q0 + 512],
                            start=True,
                            stop=True,
                        )
                    # filler before the lagged PV: keeps the PE stream moving
                    # while pso (at unit entry) or exp results are pending
                    f = fills.get(s)
                    if f is not None:
                        f()
                    # lagged PV: PE consumes pt(s-1) while ACT runs exp(s)
                    if pend is not None:
                        emit_pv(*pend)
                    # one exp over both heads' banks
                    nc.scalar.activation(pt[:, :, w:512], ps[:, :, w:512], Exp, scale=0.125)
                    # causal mask on diagonal tiles
                    if m >= 0:
                        for hh in range(2):
                            nc.vector.tensor_mul(
                                pt[:, hh, w:512],
                                pt[:, hh, w:512],
                                mask_sb[:, m, w:512],
                            )
                    pend = (i, jj, w, pt)
                emit_pv(*pend)
                # normalize and write O^T to the attn buffer; jj-major so
                # the jj=0 span's writes land first (out tiles of that span
                # unblock before the jj=1 chains finish)
                dens = []
                for hh in range(2):
                    den = denp.tile([1, 2, 512], F32, name="den")
                    nc.scalar.copy(den, pso[hh][D : D + 1, :, :])
                    dens.append(den)
                for jj in range(2):
                    for hh in range(2):
                        rec = small.tile([1, 512], F32, name="rec")
                        rb = small.tile([64, 512], F32, name="rb")
                        nc.vector.reciprocal_approx_fast(out=rec, in_=dens[hh][:, jj, :])
                        nc.gpsimd.partition_broadcast(rb, rec)
                        nc.vector.tensor_mul(
                            at_sb[po][
                                64 * hh : 64 * hh + 64,
                                qbase + jj * 512 : qbase + (jj + 1) * 512,
                            ],
                            pso[hh][0:D, jj, :],
                            rb,
                        )

            # ---- emission: lead-in projections, then attention with fillers
            for t in range(8):
                v_group(t, "s")
            qk_group(0, 0, "s")
            qk_group(2, 0, "s")
            # U1 = heads 0,1 @ q 0..1023 (12 steps)
            attn_unit(
                0, 0,
                [(0, lambda: qk_group(1, 0, "v")), (1, lambda: qk_group(3, 0, "v"))]
                + [(6 + j, lambda t=t: v_group(t, "v")) for j, t in enumerate(range(8, 12))],
            )
            # U2 = heads 2,3 @ q 0..1023 (12 steps)
            attn_unit(
                1, 0,
                [(0, lambda: qk_group(0, 1, "v")), (1, lambda: qk_group(2, 1, "v"))]
                + [(5 + j, lambda t=t: v_group(t, "v")) for j, t in enumerate(range(12, 16))],
            )
            # U3 = heads 0,1 @ q 1024..2047 (28 steps)
            attn_unit(
                0, 1,
                [(0, lambda: qk_group(1, 1, "v")), (1, lambda: qk_group(3, 1, "v"))]
                + [(8 + 4 * j, lambda t=t: out_tile(t, "v")) for j, t in enumerate(range(0, 4))],
            )
            # U4 = heads 2,3 @ q 1024..2047 (28 steps)
            attn_unit(
                1, 1,
                [(0, lambda: out_tile(4, "v")), (1, lambda: out_tile(5, "v")),
                 (12, lambda: out_tile(6, "v")), (22, lambda: out_tile(7, "v"))],
            )
            for t in range(8, NT):
                out_tile(t, "s" if t % 2 else "v")

    nc.compile()
    return nc


def _get_prog():
    global _PROG
    if _PROG is None:
        _PROG = _build()
    return _PROG


def _masks_np():
    kk = np.arange(P)[:, None]
    qq = np.arange(512)[None, :]
    return np.stack(
        [((128 * m + kk) <= qq) for m in range(4)], axis=1
    ).astype(ml_dtypes.bfloat16)


def _bf(a):
    return np.ascontiguousarray(a).astype(ml_dtypes.bfloat16)


def _shard(x, w_qkv, w_proj):
    masks = _masks_np()
    in_maps = []
    for core in range(NCORES):
        b, g = core // HPC, core % HPC
        c0 = g * QC
        in_maps.append(
            {
                "xt": _bf(x[b].T),
                "wqk": _bf(
                    np.concatenate(
                        [w_qkv[:, c0 : c0 + QC], w_qkv[:, E + c0 : E + c0 + QC]],
                        axis=1,
                    )
                ),
                "wv": _bf(w_qkv[:, 2 * E + c0 : 2 * E + c0 + QC]),
                "wp": _bf(w_proj[c0 : c0 + QC, :]),
                "masks": masks,
            }
        )
    return in_maps


def _run(inputs, **kwargs):
    x = np.asarray(inputs["x"], dtype=np.float32)
    w_qkv = np.asarray(inputs["w_qkv"], dtype=np.float32)
    w_proj = np.asarray(inputs["w_proj"], dtype=np.float32)
    b_proj = np.asarray(inputs["b_proj"], dtype=np.float32)

    nc = _get_prog()
    in_maps = _shard(x, w_qkv, w_proj)
    res = run_bass_kernel_spmd(nc, in_maps, core_ids=list(range(NCORES)), **kwargs)

    out = np.zeros((B, T, E), dtype=np.float32)
    for core in range(NCORES):
        out[core // HPC] += np.asarray(res.results[core]["out"], dtype=np.float32)
    out += b_proj[None, None, :]
    return out, res


def kernel(**inputs):
    out, _ = _run(inputs)
    return out
